# revision 30
# baseline (speedup 1.0000x reference)
"""Trainium2 Bass kernel for the MessagePassingBlock (SCNN-style 4-direction
message passing).

Math (per direction):  h[0] = x[0];  h[i] = x[i] + relu(conv1d(h[i-1]) + b)
with a [C=128 -> C=128, K=9, 'same'] conv along the non-scan spatial axis.
Output = x + sum of the 4 directional scan results.

Sharding (8 cores, SPMD — one program, per-core data):
  core c (c%4 = pair index p, batches 2p..2p+1):
    phase 1 (H-type scan, 90 steps, conv width 160):
        c < 4: 'up' scan;  c >= 4: 'down' scan (input flipped along H on host)
    phase 2 (W-type scan, 160 steps, conv width 90, input transposed on host):
        c < 4: 'left' scan; c >= 4: 'right' scan (flipped along W on host)
  Each phase fuses its 2 batch elements in the matmul free dim (320 / 180).
  The two chains are independent and interleaved so each hides the other's
  serial (PSUM->ACT->DVE) latency.

Per step: 10 matmuls — a K=1 bias tap (bias row x ones row, start=True, covers
every PSUM column) then 9 conv taps (one per tap, shifted contiguous APs over
a zero-gap-padded h row), bf16 inputs / fp32 PSUM accumulate — then ONE DVE
scalar_tensor_tensor: h[i] = bf16(max(psum, 0) + x[i]), which is also the
scan output row (y[i] == h[i]). h rows live in a 16-slot staging ring; x-in
and y-out are DMA'd 8 rows per instruction (HWDGE cost is per-instruction).
"""

import os
import numpy as np
import ml_dtypes

BF16 = ml_dtypes.bfloat16

_C = 128
_K = 9
_PAD = 4
_NCORES = 8

# Full-problem geometry (hardcoded; the harness calls kernel() with these).
_B, _H, _W = 8, 90, 160
_U = 2  # batch elements fused per core per phase

# Set by test.py to collect profile info; harmless when unset.
TRACE = False
TRACE_DIR = None
LAST_RESULTS = None

_NC_CACHE = {}


def _build_nc(Lscan1, Lconv1, Lscan2, Lconv2, U, enable_asserts=False, reps=1):
    import concourse.bass as bass
    import concourse.bacc as bacc
    import concourse.mybir as mybir
    import concourse.tile as tile
    from contextlib import ExitStack

    f32 = mybir.dt.float32
    bf16 = mybir.dt.bfloat16
    Relu = mybir.ActivationFunctionType.Relu

    nc = bacc.Bacc(
        trn_type="TRN2",
        target_bir_lowering=False,
        debug=False,
        enable_asserts=enable_asserts,
    )

    x1 = nc.dram_tensor("x1", [_C, U, Lscan1, Lconv1], bf16, kind="ExternalInput").ap()
    w1 = nc.dram_tensor("w1t", [_C, _K * _C], bf16, kind="ExternalInput").ap()
    b1 = nc.dram_tensor("b1", [1, _C], bf16, kind="ExternalInput").ap()
    x2 = nc.dram_tensor("x2", [_C, U, Lscan2, Lconv2], bf16, kind="ExternalInput").ap()
    w2 = nc.dram_tensor("w2t", [_C, _K * _C], bf16, kind="ExternalInput").ap()
    b2 = nc.dram_tensor("b2", [1, _C], bf16, kind="ExternalInput").ap()
    y1 = nc.dram_tensor("y1", [_C, U, Lscan1, Lconv1], bf16, kind="ExternalOutput").ap()
    y2 = nc.dram_tensor("y2", [_C, U, Lscan2, Lconv2], bf16, kind="ExternalOutput").ap()

    with ExitStack() as ctx:
        tc = ctx.enter_context(tile.TileContext(nc))
        const = ctx.enter_context(tc.tile_pool(name="const", bufs=1))
        xpool = ctx.enter_context(tc.tile_pool(name="xin", bufs=2))
        hpool = ctx.enter_context(tc.tile_pool(name="hstate", bufs=1))
        pspool = ctx.enter_context(tc.tile_pool(name="ps", bufs=2, space="PSUM"))

        NH = 16  # h staging ring slots (row i -> slot i % NH)
        RB = 8   # scan rows per batched x-in / y-out DMA

        Pmax = 4 + U * (max(Lconv1, Lconv2) + 4)
        ones = const.tile([1, Pmax], bf16, name="ones", tag="ones")
        nc.gpsimd.memset(ones[:], 1.0)

        def chain(tag, xD, yD, wD, bD, Lscan, Lconv):
            # h rows live in a 16-slot staging ring of padded rows: each row
            # has 4 zero columns at each end plus a shared 4-zero gap between
            # the U batch units, so every conv tap (|d| <= 4) is one fully-
            # contiguous matmul over the row and 'same'-padding zeros come
            # for free. The scan output y[i] IS h[i] (bf16), so finished rows
            # are DMA'd out straight from the ring, 8 rows per DMA (HWDGE
            # overhead is per-instruction). x rows stream in 8 per DMA too.
            # The per-step tail is two back-to-back DVE ops (relu+bias via
            # tensor_scalar, then +x), no cross-engine ACT hop.
            N = U * Lconv
            S = Lconv + 4  # unit stride within a padded row
            P = 4 + U * S  # padded row length
            wT = const.tile([_C, _K * _C], bf16, name=f"w_{tag}", tag=f"w_{tag}")
            nc.sync.dma_start(wT[:], wD[:])
            bt = const.tile([1, _C], bf16, name=f"b_{tag}", tag=f"b_{tag}")
            nc.sync.dma_start(bt[:], bD[:])

            hstage = hpool.tile(
                [_C, NH * P], bf16, name=f"hst_{tag}", tag=f"hst_{tag}"
            )
            nc.gpsimd.memset(hstage[:], 0.0)
            hs3 = hstage[:].rearrange("p (r q) -> p r q", r=NH)  # [C, NH, P]

            def row_real(slot):  # [C, U, Lconv] view of a row's real columns
                return hs3[:, slot, 4:].rearrange("p (u s) -> p u s", u=U)[
                    :, :, 0:Lconv
                ]

            # h[0] = x[0]: DMA straight into the real columns of slot 0
            nc.sync.dma_start(row_real(0), xD[:, :, 0, :])
            yield

            # batches are slot-aligned: [1..RB-1], [RB..2RB-1], ...
            def batch_of(i):
                return (i // RB) * RB

            # tap d=0 first: it covers every output column, so start=True on it
            # clears/overwrites the full PSUM region before other taps accumulate.
            order = [0] + [d for d in range(-_PAD, _PAD + 1) if d != 0]
            xt3 = None
            for i in range(1, Lscan):
                b0 = max(1, batch_of(i))
                bend = min(Lscan - 1, batch_of(i) + RB - 1)
                if i == b0:  # batched x prefetch for rows b0..bend
                    n = bend - b0 + 1
                    xt = xpool.tile(
                        [_C, U, RB, Lconv], bf16, name=f"x_{tag}", tag=f"x_{tag}"
                    )
                    xt3 = xt[:]
                    nc.sync.dma_start(xt3[:, :, 0:n, :], xD[:, :, b0 : b0 + n, :])
                sp = (i - 1) % NH
                si = i % NH
                ps = pspool.tile([_C, P], f32, name=f"ps_{tag}", tag=f"ps_{tag}")
                # bias tap first: K=1 matmul (bias row x ones row) writes every
                # PSUM column with start=True, so conv taps just accumulate.
                nc.tensor.matmul(
                    ps[:, 0:P], bt[0:1, :], ones[0:1, 0:P], start=True, stop=False
                )
                for j, d in enumerate(order):
                    k = d + _PAD
                    a = max(d, 0)
                    bo = max(-d, 0)
                    ln = P - abs(d)
                    nc.tensor.matmul(
                        ps[:, bo : bo + ln],
                        wT[:, k * _C : (k + 1) * _C],
                        hs3[:, sp, a : a + ln],
                        start=False,
                        stop=(j == len(order) - 1),
                    )
                ps3 = ps[:, 4:].rearrange("p (u s) -> p u s", u=U)[:, :, 0:Lconv]
                # h[i] = bf16(relu(psum) + x[i]) in ONE DVE op — also the
                # scan output row
                nc.vector.scalar_tensor_tensor(
                    row_real(si), ps3, 0.0, xt3[:, :, i - b0, :],
                    mybir.AluOpType.max, mybir.AluOpType.add,
                )
                if i == bend:  # batched y writeback for rows b0..i (per unit)
                    n = i - b0 + 1
                    s0 = b0 % NH
                    for u in range(U):
                        src = hs3[:, s0 : s0 + n, 4 + u * S : 4 + u * S + Lconv]
                        nc.sync.dma_start(yD[:, u, b0 : b0 + n, :], src)
                yield

        for _rep in range(reps):
            c1 = chain("h", x1, y1, w1, b1, Lscan1, Lconv1)
            c2 = chain("w", x2, y2, w2, b2, Lscan2, Lconv2)
            # Interleave the two chains proportionally so both finish together
            # and each one's matmuls fill the other's post-matmul latency.
            t1 = t2 = 0
            done1 = done2 = False
            while not (done1 and done2):
                if not done1 and (done2 or t1 * Lscan2 <= t2 * Lscan1):
                    try:
                        next(c1)
                        t1 += 1
                    except StopIteration:
                        done1 = True
                elif not done2:
                    try:
                        next(c2)
                        t2 += 1
                    except StopIteration:
                        done2 = True
                else:
                    done2 = True
    nc.compile()
    return nc


def _get_nc():
    key = (_H, _W, _U)
    if key not in _NC_CACHE:
        _NC_CACHE[key] = _build_nc(_H, _W, _W, _H, _U)
    return _NC_CACHE[key]


def _prep_w(w):
    # w: [O, I, K] -> lhsT layout [I, K*O] with lhsT[i, k*128+o] = w[o, i, k]
    return np.ascontiguousarray(
        np.transpose(np.asarray(w, np.float32), (1, 2, 0)).reshape(_C, _K * _C)
    ).astype(BF16)


def _make_in_maps(x, inputs):
    w1t = {0: _prep_w(inputs["up_w"]), 1: _prep_w(inputs["down_w"])}
    w2t = {0: _prep_w(inputs["left_w"]), 1: _prep_w(inputs["right_w"])}
    b1v = {0: np.asarray(inputs["up_b"], np.float32).reshape(1, _C).astype(BF16),
           1: np.asarray(inputs["down_b"], np.float32).reshape(1, _C).astype(BF16)}
    b2v = {0: np.asarray(inputs["left_b"], np.float32).reshape(1, _C).astype(BF16),
           1: np.asarray(inputs["right_b"], np.float32).reshape(1, _C).astype(BF16)}

    in_maps = []
    for c in range(_NCORES):
        p = c % 4
        g = c // 4  # 0: up/left, 1: down/right
        xb = x[2 * p : 2 * p + 2]  # [2, C, H, W]
        x1 = xb.transpose(1, 0, 2, 3)  # [C, 2, H, W]
        if g:
            x1 = x1[:, :, ::-1, :]
        x2 = xb.transpose(1, 0, 3, 2)  # [C, 2, W, H]
        if g:
            x2 = x2[:, :, ::-1, :]
        in_maps.append(
            {
                "x1": np.ascontiguousarray(x1).astype(BF16),
                "w1t": w1t[g],
                "b1": b1v[g],
                "x2": np.ascontiguousarray(x2).astype(BF16),
                "w2t": w2t[g],
                "b2": b2v[g],
            }
        )
    return in_maps


def kernel(x, up_w, up_b, down_w, down_b, left_w, left_b, right_w, right_b):
    global LAST_RESULTS
    from concourse.bass_utils import run_bass_kernel_spmd

    x = np.asarray(x, np.float32)
    assert x.shape == (_B, _C, _H, _W)

    nc = _get_nc()
    in_maps = _make_in_maps(
        x,
        dict(up_w=up_w, up_b=up_b, down_w=down_w, down_b=down_b,
             left_w=left_w, left_b=left_b, right_w=right_w, right_b=right_b),
    )

    res = run_bass_kernel_spmd(
        nc,
        in_maps,
        list(range(_NCORES)),
        trace=TRACE,
        tmpdir=TRACE_DIR,
    )
    LAST_RESULTS = res

    out = x.copy()
    for c in range(_NCORES):
        p = c % 4
        g = c // 4
        bsl = slice(2 * p, 2 * p + 2)
        xb = x[bsl]  # [2, C, H, W] fp32
        y1 = np.array(res.results[c]["y1"]).astype(np.float32)  # [C,2,H,W] scan layout
        # h[0] = x[0] exactly (fp32 on the host, not the bf16-rounded copy)
        y1[:, :, 0, :] = xb[:, :, _H - 1 if g else 0, :].transpose(1, 0, 2)
        if g:
            y1 = y1[:, :, ::-1, :]
        out[bsl] += y1.transpose(1, 0, 2, 3)
        y2 = np.array(res.results[c]["y2"]).astype(np.float32)  # [C,2,W,H] scan layout
        y2[:, :, 0, :] = xb[:, :, :, _W - 1 if g else 0].transpose(1, 0, 2)
        if g:
            y2 = y2[:, :, ::-1, :]
        out[bsl] += y2.transpose(1, 0, 3, 2)
    return out


# ---------------------------------------------------------------------------
# Self-test: validate the emitted program in CoreSim on a tiny geometry.
# ---------------------------------------------------------------------------


def _mini_scan(xs, w, b, Lscan, Lconv, U):
    # xs: [C, U, Lscan, Lconv] bf16-valued float32; y[i] = h[i] = bf16 state.
    ys = np.zeros_like(xs)
    ys[:, :, 0, :] = xs[:, :, 0, :]
    h = xs[:, :, 0, :].copy()
    wf = np.asarray(w, np.float32).astype(BF16).astype(np.float32)
    bb = np.asarray(b, np.float32).astype(BF16).astype(np.float32)
    for i in range(1, Lscan):
        acc = np.zeros((_C, U, Lconv), np.float32) + bb[:, None, None]
        for k in range(_K):
            d = k - _PAD
            a = max(d, 0)
            bo = max(-d, 0)
            ln = Lconv - abs(d)
            if ln <= 0:
                continue
            acc[:, :, bo : bo + ln] += np.einsum(
                "oi,iul->oul", wf[:, :, k], h[:, :, a : a + ln], optimize=True
            )
        t = np.maximum(acc, 0.0)
        h = (t + xs[:, :, i, :]).astype(BF16).astype(np.float32)
        ys[:, :, i, :] = h
    return ys


def _selftest_sim():
    from concourse.bass_interp import CoreSim

    rng = np.random.default_rng(0)
    Ls1, Lc1, Ls2, Lc2, U = 5, 12, 7, 10, 2
    nc = _build_nc(Ls1, Lc1, Ls2, Lc2, U, enable_asserts=True)

    sc = 1.0 / np.sqrt(_C * _K)
    x1 = rng.standard_normal((_C, U, Ls1, Lc1)).astype(np.float32)
    x2 = rng.standard_normal((_C, U, Ls2, Lc2)).astype(np.float32)
    w1 = (rng.standard_normal((_C, _C, _K)) * sc).astype(np.float32)
    w2 = (rng.standard_normal((_C, _C, _K)) * sc).astype(np.float32)
    b1 = (rng.standard_normal(_C) * 0.01).astype(np.float32)
    b2 = (rng.standard_normal(_C) * 0.01).astype(np.float32)

    sim = CoreSim(nc, trace=False)
    sim.tensor("x1")[:] = x1.astype(BF16)
    sim.tensor("w1t")[:] = _prep_w(w1)
    sim.tensor("b1")[:] = b1.reshape(1, _C).astype(BF16)
    sim.tensor("x2")[:] = x2.astype(BF16)
    sim.tensor("w2t")[:] = _prep_w(w2)
    sim.tensor("b2")[:] = b2.reshape(1, _C).astype(BF16)
    sim.simulate(check_with_hw=False)

    for name, xs, w, b, Ls, Lc in [
        ("y1", x1, w1, b1, Ls1, Lc1),
        ("y2", x2, w2, b2, Ls2, Lc2),
    ]:
        got = np.asarray(sim.tensor(name)).astype(np.float32).copy()
        xsb = xs.astype(BF16).astype(np.float32)
        got[:, :, 0, :] = xsb[:, :, 0, :]
        exp = _mini_scan(xsb, w, b, Ls, Lc, U)
        rel = np.linalg.norm(got - exp) / np.linalg.norm(exp)
        print(f"{name}: rel l2 = {rel:.3e}  absmax = {np.abs(got - exp).max():.3e}")
        assert rel < 5e-3, f"{name} mismatch: rel {rel}"
    print("selftest sim OK")


if __name__ == "__main__":
    import sys

    if "--sim" in sys.argv:
        _selftest_sim()
